# revision 20
# baseline (speedup 1.0000x reference)
"""Trainium2 Bass kernel for nn_Attention (B=4, N=1024, E=1024, H=16).

Computation (matching the reference):
    Q = q @ Wq.T + bq ; K = k @ Wk.T + bk ; V = v @ Wv.T + bv   (per batch)
    attn = softmax(Q_h @ K_h^T / sqrt(E))  per head h           -> output 2
    out  = V_h * rowsum(attn) = V_h (softmax rows sum to 1)     -> output 1

Sharding: 8 cores = 4 batches x 2 head-halves. Core c handles batch c//2 and
heads 8*(c%2) .. 8*(c%2)+8 (a 512-wide slice of the projection features).

Per core:
  - one packed [Wt | xT] tensor per projection, loaded with casting DMAs
    (fp32 -> fp32r: full PE-rate matmuls at ~12-13 bit mantissa) in 2-k-tile
    chunks, lane-chained q->k->v so the Q pack lands at full bandwidth first,
  - Q/K projections into [feat, n] f32r SBUF tiles (bias via DVE on PSUM
    eviction),
  - per head pair / 128-row tile: scores into a 2-bank PSUM tile, one
    exp(x/32) pass on the Scalar engine with fused row-sum, reciprocal +
    normalize on the Vector engine, one 1MB DMA out,
  - V projection split across the last two head pairs, bias via broadcast
    add, DMA out.
"""

import numpy as np

import concourse.bass as bass
import concourse.mybir as mybir
import concourse.tile as tile
from concourse import bacc
from concourse.bass_utils import run_bass_kernel_spmd
from concourse.tile_rust import add_dep_helper

B, N, E, H = 4, 1024, 1024, 16
D = E // H            # 64
NCORES = 8
FEAT = E // 2         # 512 projection features per core
HPC = H // 2          # 8 heads per core
KT = E // 128         # 8 contraction tiles
SCALE = float(E) ** 0.5

F32 = mybir.dt.float32
F32R = mybir.dt.float32r
AF = mybir.ActivationFunctionType

_cached_nc = None


def _build():
    nc = bacc.Bacc("TRN2", target_bir_lowering=False, debug=False)
    pq = nc.declare_dram_parameter("pq", [E, FEAT + N], F32, isOutput=False)
    pk = nc.declare_dram_parameter("pk", [E, FEAT + N], F32, isOutput=False)
    pv = nc.declare_dram_parameter("pv", [E, FEAT + N], F32, isOutput=False)
    bqk = nc.declare_dram_parameter("bqk", [128, 8], F32, isOutput=False)
    bvr = nc.declare_dram_parameter("bvr", [FEAT], F32, isOutput=False)
    attn = nc.declare_dram_parameter("attn", [HPC, N, N], F32, isOutput=True)
    o = nc.declare_dram_parameter("o", [N, FEAT], F32, isOutput=True)

    with tile.TileContext(nc) as tc:
        with tc.tile_pool(name="packs", bufs=2) as packp, \
             tc.tile_pool(name="proj", bufs=8) as projp, \
             tc.tile_pool(name="vout", bufs=3) as voutp, \
             tc.tile_pool(name="attnp", bufs=4) as attnp, \
             tc.tile_pool(name="small", bufs=1) as smallp, \
             tc.tile_pool(name="sums", bufs=12) as sumsp, \
             tc.tile_pool(name="psumproj", bufs=1, space="PSUM") as psump, \
             tc.tile_pool(name="psumsc", bufs=3, space="PSUM") as psumsc:

            bqk_t = smallp.tile([128, 8], F32, tag="bqk")
            nc.sync.dma_start(out=bqk_t, in_=bqk[:, :])
            bv_t = smallp.tile([128, FEAT], F32, tag="bv")
            bvr_ap = bvr[:]
            bv_bcast = bass.AP(
                tensor=bvr_ap.tensor,
                offset=bvr_ap.offset,
                ap=[[0, 128]] + [list(x) for x in bvr_ap.ap],
            )
            nc.sync.dma_start(out=bv_t, in_=bv_bcast)

            # Chained casting pack loads, 2-k-tile chunks: chunks of one pack
            # stream concurrently; pack i's chunk c waits on pack i-1's
            # chunk c, so earlier packs get full aggregate bandwidth.
            CH = 2
            chunk_dmas = {}

            def load_pack(pi, param):
                t = packp.tile([128, KT, FEAT + N], F32R, tag="pack")
                src = param.rearrange("(k p) m -> p k m", p=128)
                for ci, c0 in enumerate(range(0, KT, CH)):
                    d = nc.gpsimd.dma_start(
                        out=t[:, c0:c0 + CH, :], in_=src[:, c0:c0 + CH, :])
                    if pi > 0:
                        add_dep_helper(d.ins, chunk_dmas[(pi - 1, ci)].ins,
                                       reason="pack order q->k->v per lane")
                    chunk_dmas[(pi, ci)] = d
                return t

            pq_t = load_pack(0, pq)
            pk_t = load_pack(1, pk)
            pv_t = load_pack(2, pv)

            # Q/K projections -> [feat, n] f32r tiles with bias folded in.
            qk_tiles = {}

            def qk_projection(t, f):
                pack_t, bcol = [(pq_t, 0), (pk_t, 4)][t]
                pt_out = projp.tile([128, N], F32R,
                                    tag=("qproj" if t == 0 else "kproj"))
                ps = psump.tile([128, N], F32, tag="proj")
                for nh in range(2):
                    for k in range(KT):
                        nc.tensor.matmul(
                            ps[:, 512 * nh:512 * (nh + 1)],
                            pack_t[:, k, 128 * f:128 * (f + 1)],
                            pack_t[:, k, FEAT + 512 * nh:FEAT + 512 * (nh + 1)],
                            start=(k == 0),
                            stop=(k == KT - 1),
                        )
                nc.vector.tensor_scalar_add(
                    out=pt_out, in0=ps,
                    scalar1=bqk_t[:, bcol + f:bcol + f + 1],
                )
                qk_tiles[(t, f)] = pt_out

            def v_projection(nts):
                for nt in nts:
                    vp = voutp.tile([128, FEAT], F32, tag="vproj")
                    ps = psump.tile([128, 512], F32, tag="proj")
                    for k in range(KT):
                        nc.tensor.matmul(
                            ps,
                            pv_t[:, k, FEAT + 128 * nt:FEAT + 128 * (nt + 1)],
                            pv_t[:, k, 0:FEAT],
                            start=(k == 0),
                            stop=(k == KT - 1),
                        )
                    nc.vector.scalar_tensor_tensor(
                        out=vp, in0=ps, scalar=1.0, in1=bv_t,
                        op0=mybir.AluOpType.mult, op1=mybir.AluOpType.add)
                    nc.sync.dma_start(
                        out=o[128 * nt:128 * (nt + 1), :], in_=vp)

            # Scores + softmax per head pair j (local heads 2j, 2j+1).
            # All Q projections run upfront (only need the first pack); the
            # K projection for pair j is issued right before pair j's scores
            # so softmax starts as soon as K f0 is ready instead of after
            # all of K.  16-matmul K blocks hide in the softmax backlog.
            inv_scale = 1.0 / SCALE
            for f in range(FEAT // 128):
                qk_projection(0, f)
            for j in range(4):
                qk_projection(1, j)
                if j == 2:
                    v_projection(range(0, 4))
                elif j == 3:
                    v_projection(range(4, 8))
                qt = qk_tiles[(0, j)]
                kt_ = qk_tiles[(1, j)]
                for m in range(8):
                    at_t = attnp.tile([128, 2 * N], F32, tag="attn")
                    sm = sumsp.tile([128, 4], F32, tag="sums")
                    for hh in range(2):
                        lhsT = qt[64 * hh:64 * (hh + 1), 128 * m:128 * (m + 1)]
                        ps = psumsc.tile([128, N], F32, tag="sc")
                        for lh in range(2):
                            nc.tensor.matmul(
                                ps[:, 512 * lh:512 * (lh + 1)], lhsT,
                                kt_[64 * hh:64 * (hh + 1), 512 * lh:512 * (lh + 1)],
                                start=True, stop=True,
                            )
                        nc.scalar.activation(
                            out=at_t[:, N * hh:N * (hh + 1)],
                            in_=ps,
                            func=AF.Exp,
                            scale=inv_scale,
                            accum_out=sm[:, hh:hh + 1],
                        )
                    for hh in range(2):
                        nc.vector.reciprocal(sm[:, 2 + hh:3 + hh],
                                             sm[:, hh:hh + 1])
                        nc.vector.tensor_scalar_mul(
                            at_t[:, N * hh:N * (hh + 1)],
                            at_t[:, N * hh:N * (hh + 1)],
                            sm[:, 2 + hh:3 + hh])
                    out_ap = attn[2 * j:2 * j + 2,
                                  128 * m:128 * (m + 1), :].transpose([1, 0, 2])
                    nc.sync.dma_start(
                        out=out_ap,
                        in_=at_t[:].rearrange("p (h n) -> p h n", h=2))

    nc.compile()
    return nc


def make_in_maps(q, k, v, Wq, bq, Wk, bk, Wv, bv):
    q, k, v = (np.asarray(x, np.float32) for x in (q, k, v))
    Wq, Wk, Wv = (np.asarray(x, np.float32) for x in (Wq, Wk, Wv))
    bq, bk, bv = (np.asarray(x, np.float32) for x in (bq, bk, bv))

    xT = {}
    for b in range(B):
        xT[("q", b)] = np.ascontiguousarray(q[b].T)
        xT[("k", b)] = np.ascontiguousarray(k[b].T)
        xT[("v", b)] = np.ascontiguousarray(v[b].T)
    WT = {}
    for s in range(2):
        WT[("q", s)] = np.ascontiguousarray(Wq[FEAT * s:FEAT * (s + 1), :].T)
        WT[("k", s)] = np.ascontiguousarray(Wk[FEAT * s:FEAT * (s + 1), :].T)
        WT[("v", s)] = np.ascontiguousarray(Wv[FEAT * s:FEAT * (s + 1), :].T)

    in_maps = []
    for c in range(NCORES):
        b, s = divmod(c, 2)
        m = {}
        for t, name in (("q", "pq"), ("k", "pk"), ("v", "pv")):
            m[name] = np.concatenate([WT[(t, s)], xT[(t, b)]], axis=1)
        bqk_arr = np.empty((128, 8), np.float32)
        for f in range(4):
            bqk_arr[:, f] = bq[FEAT * s + 128 * f:FEAT * s + 128 * (f + 1)]
            bqk_arr[:, 4 + f] = bk[FEAT * s + 128 * f:FEAT * s + 128 * (f + 1)]
        m["bqk"] = bqk_arr
        m["bvr"] = bv[FEAT * s:FEAT * (s + 1)].copy()
        in_maps.append(m)
    return in_maps


def gather(results):
    out = np.empty((B, H, N, D), np.float32)
    attn = np.empty((B, H, N, N), np.float32)
    for c in range(NCORES):
        b, s = divmod(c, 2)
        attn[b, HPC * s:HPC * (s + 1)] = results[c]["attn"]
        out[b, HPC * s:HPC * (s + 1)] = (
            results[c]["o"].reshape(N, HPC, D).transpose(1, 0, 2))
    return out, attn


def get_nc():
    global _cached_nc
    if _cached_nc is None:
        _cached_nc = _build()
    return _cached_nc


def kernel(q, k, v, Wq, bq, Wk, bk, Wv, bv):
    nc = get_nc()
    in_maps = make_in_maps(q, k, v, Wq, bq, Wk, bk, Wv, bv)
    res = run_bass_kernel_spmd(nc, in_maps, list(range(NCORES)))
    return gather(res.results)


# revision 21
# speedup vs baseline: 1.0800x; 1.0800x over previous
"""Trainium2 Bass kernel for nn_Attention (B=4, N=1024, E=1024, H=16).

Computation (matching the reference):
    Q = q @ Wq.T + bq ; K = k @ Wk.T + bk ; V = v @ Wv.T + bv   (per batch)
    attn = softmax(Q_h @ K_h^T / sqrt(E))  per head h           -> output 2
    out  = V_h * rowsum(attn) = V_h (softmax rows sum to 1)     -> output 1

Sharding: 8 cores = 4 batches x 2 head-halves. Core c handles batch c//2 and
heads 8*(c%2) .. 8*(c%2)+8 (a 512-wide slice of the projection features).

Per core:
  - one packed [Wt | xT] tensor per projection, loaded with casting DMAs
    (fp32 -> fp32r: full PE-rate matmuls at ~12-13 bit mantissa) in 2-k-tile
    chunks, lane-chained q->k->v so the Q pack lands at full bandwidth first,
  - Q/K projections into [feat, n] f32r SBUF tiles (bias via DVE on PSUM
    eviction),
  - per head pair / 128-row tile: scores into a 2-bank PSUM tile, one
    exp(x/32) pass on the Scalar engine with fused row-sum, reciprocal +
    normalize on the Vector engine, one 1MB DMA out,
  - V projection split across the last two head pairs, bias via broadcast
    add, DMA out.
"""

import numpy as np

import concourse.bass as bass
import concourse.mybir as mybir
import concourse.tile as tile
from concourse import bacc
from concourse.bass_utils import run_bass_kernel_spmd
from concourse.tile_rust import add_dep_helper

B, N, E, H = 4, 1024, 1024, 16
D = E // H            # 64
NCORES = 8
FEAT = E // 2         # 512 projection features per core
HPC = H // 2          # 8 heads per core
KT = E // 128         # 8 contraction tiles
SCALE = float(E) ** 0.5

F32 = mybir.dt.float32
F32R = mybir.dt.float32r
AF = mybir.ActivationFunctionType

_cached_nc = None


def _build():
    nc = bacc.Bacc("TRN2", target_bir_lowering=False, debug=False)
    pq = nc.declare_dram_parameter("pq", [E, FEAT + N], F32, isOutput=False)
    pk = nc.declare_dram_parameter("pk", [E, FEAT + N], F32, isOutput=False)
    pv = nc.declare_dram_parameter("pv", [E, FEAT + N], F32, isOutput=False)
    bqk = nc.declare_dram_parameter("bqk", [128, 8], F32, isOutput=False)
    bvr = nc.declare_dram_parameter("bvr", [FEAT], F32, isOutput=False)
    attn = nc.declare_dram_parameter("attn", [HPC, N, N], F32, isOutput=True)
    o = nc.declare_dram_parameter("o", [N, FEAT], F32, isOutput=True)

    with tile.TileContext(nc) as tc:
        with tc.tile_pool(name="packs", bufs=2) as packp, \
             tc.tile_pool(name="proj", bufs=8) as projp, \
             tc.tile_pool(name="vout", bufs=3) as voutp, \
             tc.tile_pool(name="attnp", bufs=4) as attnp, \
             tc.tile_pool(name="small", bufs=1) as smallp, \
             tc.tile_pool(name="sums", bufs=12) as sumsp, \
             tc.tile_pool(name="psumproj", bufs=2, space="PSUM") as psump, \
             tc.tile_pool(name="psumsc", bufs=3, space="PSUM") as psumsc:

            bqk_t = smallp.tile([128, 8], F32, tag="bqk")
            nc.sync.dma_start(out=bqk_t, in_=bqk[:, :])
            bv_t = smallp.tile([128, FEAT], F32, tag="bv")
            bvr_ap = bvr[:]
            bv_bcast = bass.AP(
                tensor=bvr_ap.tensor,
                offset=bvr_ap.offset,
                ap=[[0, 128]] + [list(x) for x in bvr_ap.ap],
            )
            nc.sync.dma_start(out=bv_t, in_=bv_bcast)

            # Chained casting pack loads, 2-k-tile chunks: chunks of one pack
            # stream concurrently; pack i's chunk c waits on pack i-1's
            # chunk c, so earlier packs get full aggregate bandwidth.
            CH = 2
            chunk_dmas = {}

            def load_pack(pi, param):
                t = packp.tile([128, KT, FEAT + N], F32R, tag="pack")
                src = param.rearrange("(k p) m -> p k m", p=128)
                for ci, c0 in enumerate(range(0, KT, CH)):
                    d = nc.gpsimd.dma_start(
                        out=t[:, c0:c0 + CH, :], in_=src[:, c0:c0 + CH, :])
                    if pi > 0:
                        add_dep_helper(d.ins, chunk_dmas[(pi - 1, ci)].ins,
                                       reason="pack order q->k->v per lane")
                    chunk_dmas[(pi, ci)] = d
                return t

            pq_t = load_pack(0, pq)
            pk_t = load_pack(1, pk)
            pv_t = load_pack(2, pv)

            # Q/K projections -> [feat, n] f32r tiles with bias folded in.
            qk_tiles = {}

            def qk_projection(t, f):
                pack_t, bcol = [(pq_t, 0), (pk_t, 4)][t]
                pt_out = projp.tile([128, N], F32R,
                                    tag=("qproj" if t == 0 else "kproj"))
                for nh in range(2):
                    ps = psump.tile([128, 512], F32, tag="proj")
                    for k in range(KT):
                        nc.tensor.matmul(
                            ps,
                            pack_t[:, k, 128 * f:128 * (f + 1)],
                            pack_t[:, k, FEAT + 512 * nh:FEAT + 512 * (nh + 1)],
                            start=(k == 0),
                            stop=(k == KT - 1),
                        )
                    nc.vector.tensor_scalar_add(
                        out=pt_out[:, 512 * nh:512 * (nh + 1)], in0=ps,
                        scalar1=bqk_t[:, bcol + f:bcol + f + 1],
                    )
                qk_tiles[(t, f)] = pt_out

            def v_projection(nts):
                for nt in nts:
                    vp = voutp.tile([128, FEAT], F32, tag="vproj")
                    ps = psump.tile([128, 512], F32, tag="proj")
                    for k in range(KT):
                        nc.tensor.matmul(
                            ps,
                            pv_t[:, k, FEAT + 128 * nt:FEAT + 128 * (nt + 1)],
                            pv_t[:, k, 0:FEAT],
                            start=(k == 0),
                            stop=(k == KT - 1),
                        )
                    nc.vector.scalar_tensor_tensor(
                        out=vp, in0=ps, scalar=1.0, in1=bv_t,
                        op0=mybir.AluOpType.mult, op1=mybir.AluOpType.add)
                    nc.sync.dma_start(
                        out=o[128 * nt:128 * (nt + 1), :], in_=vp)

            # Scores + softmax per head pair j (local heads 2j, 2j+1).
            # All Q projections run upfront (only need the first pack); the
            # K projection for pair j is issued right before pair j's scores
            # so softmax starts as soon as K f0 is ready instead of after
            # all of K.  16-matmul K blocks hide in the softmax backlog.
            inv_scale = 1.0 / SCALE
            for f in range(FEAT // 128):
                qk_projection(0, f)
            for j in range(4):
                qk_projection(1, j)
                if j == 2:
                    v_projection(range(0, 4))
                elif j == 3:
                    v_projection(range(4, 8))
                qt = qk_tiles[(0, j)]
                kt_ = qk_tiles[(1, j)]
                for m in range(8):
                    at_t = attnp.tile([128, 2 * N], F32, tag="attn")
                    sm = sumsp.tile([128, 4], F32, tag="sums")
                    for hh in range(2):
                        lhsT = qt[64 * hh:64 * (hh + 1), 128 * m:128 * (m + 1)]
                        ps = psumsc.tile([128, N], F32, tag="sc")
                        for lh in range(2):
                            nc.tensor.matmul(
                                ps[:, 512 * lh:512 * (lh + 1)], lhsT,
                                kt_[64 * hh:64 * (hh + 1), 512 * lh:512 * (lh + 1)],
                                start=True, stop=True,
                            )
                        nc.scalar.activation(
                            out=at_t[:, N * hh:N * (hh + 1)],
                            in_=ps,
                            func=AF.Exp,
                            scale=inv_scale,
                            accum_out=sm[:, hh:hh + 1],
                        )
                    for hh in range(2):
                        nc.vector.reciprocal(sm[:, 2 + hh:3 + hh],
                                             sm[:, hh:hh + 1])
                        nc.vector.tensor_scalar_mul(
                            at_t[:, N * hh:N * (hh + 1)],
                            at_t[:, N * hh:N * (hh + 1)],
                            sm[:, 2 + hh:3 + hh])
                    out_ap = attn[2 * j:2 * j + 2,
                                  128 * m:128 * (m + 1), :].transpose([1, 0, 2])
                    nc.sync.dma_start(
                        out=out_ap,
                        in_=at_t[:].rearrange("p (h n) -> p h n", h=2))

    nc.compile()
    return nc


def make_in_maps(q, k, v, Wq, bq, Wk, bk, Wv, bv):
    q, k, v = (np.asarray(x, np.float32) for x in (q, k, v))
    Wq, Wk, Wv = (np.asarray(x, np.float32) for x in (Wq, Wk, Wv))
    bq, bk, bv = (np.asarray(x, np.float32) for x in (bq, bk, bv))

    xT = {}
    for b in range(B):
        xT[("q", b)] = np.ascontiguousarray(q[b].T)
        xT[("k", b)] = np.ascontiguousarray(k[b].T)
        xT[("v", b)] = np.ascontiguousarray(v[b].T)
    WT = {}
    for s in range(2):
        WT[("q", s)] = np.ascontiguousarray(Wq[FEAT * s:FEAT * (s + 1), :].T)
        WT[("k", s)] = np.ascontiguousarray(Wk[FEAT * s:FEAT * (s + 1), :].T)
        WT[("v", s)] = np.ascontiguousarray(Wv[FEAT * s:FEAT * (s + 1), :].T)

    in_maps = []
    for c in range(NCORES):
        b, s = divmod(c, 2)
        m = {}
        for t, name in (("q", "pq"), ("k", "pk"), ("v", "pv")):
            m[name] = np.concatenate([WT[(t, s)], xT[(t, b)]], axis=1)
        bqk_arr = np.empty((128, 8), np.float32)
        for f in range(4):
            bqk_arr[:, f] = bq[FEAT * s + 128 * f:FEAT * s + 128 * (f + 1)]
            bqk_arr[:, 4 + f] = bk[FEAT * s + 128 * f:FEAT * s + 128 * (f + 1)]
        m["bqk"] = bqk_arr
        m["bvr"] = bv[FEAT * s:FEAT * (s + 1)].copy()
        in_maps.append(m)
    return in_maps


def gather(results):
    out = np.empty((B, H, N, D), np.float32)
    attn = np.empty((B, H, N, N), np.float32)
    for c in range(NCORES):
        b, s = divmod(c, 2)
        attn[b, HPC * s:HPC * (s + 1)] = results[c]["attn"]
        out[b, HPC * s:HPC * (s + 1)] = (
            results[c]["o"].reshape(N, HPC, D).transpose(1, 0, 2))
    return out, attn


def get_nc():
    global _cached_nc
    if _cached_nc is None:
        _cached_nc = _build()
    return _cached_nc


def kernel(q, k, v, Wq, bq, Wk, bk, Wv, bv):
    nc = get_nc()
    in_maps = make_in_maps(q, k, v, Wq, bq, Wk, bk, Wv, bv)
    res = run_bass_kernel_spmd(nc, in_maps, list(range(NCORES)))
    return gather(res.results)


# revision 22
# speedup vs baseline: 1.1345x; 1.0505x over previous
"""Trainium2 Bass kernel for nn_Attention (B=4, N=1024, E=1024, H=16).

Computation (matching the reference):
    Q = q @ Wq.T + bq ; K = k @ Wk.T + bk ; V = v @ Wv.T + bv   (per batch)
    attn = softmax(Q_h @ K_h^T / sqrt(E))  per head h           -> output 2
    out  = V_h * rowsum(attn) = V_h (softmax rows sum to 1)     -> output 1

Sharding: 8 cores = 4 batches x 2 head-halves. Core c handles batch c//2 and
heads 8*(c%2) .. 8*(c%2)+8 (a 512-wide slice of the projection features).

Per core:
  - one packed [Wt | xT] tensor per projection, loaded with casting DMAs
    (fp32 -> fp32r: full PE-rate matmuls at ~12-13 bit mantissa) in 2-k-tile
    chunks, lane-chained q->k->v so the Q pack lands at full bandwidth first,
  - Q/K projections into [feat, n] f32r SBUF tiles (bias via DVE on PSUM
    eviction),
  - per head pair / 128-row tile: scores into a 2-bank PSUM tile, one
    exp(x/32) pass on the Scalar engine with fused row-sum, reciprocal +
    normalize on the Vector engine, one 1MB DMA out,
  - V projection split across the last two head pairs, bias via broadcast
    add, DMA out.
"""

import numpy as np

import concourse.bass as bass
import concourse.mybir as mybir
import concourse.tile as tile
from concourse import bacc
from concourse.bass_utils import run_bass_kernel_spmd
from concourse.tile_rust import add_dep_helper

B, N, E, H = 4, 1024, 1024, 16
D = E // H            # 64
NCORES = 8
FEAT = E // 2         # 512 projection features per core
HPC = H // 2          # 8 heads per core
KT = E // 128         # 8 contraction tiles
SCALE = float(E) ** 0.5

F32 = mybir.dt.float32
F32R = mybir.dt.float32r
AF = mybir.ActivationFunctionType

_cached_nc = None


def _build():
    nc = bacc.Bacc("TRN2", target_bir_lowering=False, debug=False)
    pq = nc.declare_dram_parameter("pq", [E, FEAT + N], F32, isOutput=False)
    pk = nc.declare_dram_parameter("pk", [E, FEAT + N], F32, isOutput=False)
    pv = nc.declare_dram_parameter("pv", [E, FEAT + N], F32, isOutput=False)
    bqk = nc.declare_dram_parameter("bqk", [128, 8], F32, isOutput=False)
    bvr = nc.declare_dram_parameter("bvr", [FEAT], F32, isOutput=False)
    attn = nc.declare_dram_parameter("attn", [HPC, N, N], F32, isOutput=True)
    o = nc.declare_dram_parameter("o", [N, FEAT], F32, isOutput=True)

    with tile.TileContext(nc) as tc:
        with tc.tile_pool(name="packs", bufs=2) as packp, \
             tc.tile_pool(name="proj", bufs=8) as projp, \
             tc.tile_pool(name="vout", bufs=3) as voutp, \
             tc.tile_pool(name="attnp", bufs=4) as attnp, \
             tc.tile_pool(name="small", bufs=1) as smallp, \
             tc.tile_pool(name="sums", bufs=12) as sumsp, \
             tc.tile_pool(name="psumproj", bufs=2, space="PSUM") as psump, \
             tc.tile_pool(name="psumsc", bufs=3, space="PSUM") as psumsc:

            bqk_t = smallp.tile([128, 8], F32, tag="bqk")
            nc.sync.dma_start(out=bqk_t, in_=bqk[:, :])
            bv_t = smallp.tile([128, FEAT], F32, tag="bv")
            bvr_ap = bvr[:]
            bv_bcast = bass.AP(
                tensor=bvr_ap.tensor,
                offset=bvr_ap.offset,
                ap=[[0, 128]] + [list(x) for x in bvr_ap.ap],
            )
            nc.sync.dma_start(out=bv_t, in_=bv_bcast)

            # Chained casting pack loads, 2-k-tile chunks: chunks of one pack
            # stream concurrently; pack i's chunk c waits on pack i-1's
            # chunk c, so earlier packs get full aggregate bandwidth.
            CH = 2
            chunk_dmas = {}

            def load_pack(pi, param):
                t = packp.tile([128, KT, FEAT + N], F32R, tag="pack")
                src = param.rearrange("(k p) m -> p k m", p=128)
                for ci, c0 in enumerate(range(0, KT, CH)):
                    d = nc.gpsimd.dma_start(
                        out=t[:, c0:c0 + CH, :], in_=src[:, c0:c0 + CH, :])
                    if pi > 0:
                        add_dep_helper(d.ins, chunk_dmas[(pi - 1, ci)].ins,
                                       reason="pack order q->k->v per lane")
                    chunk_dmas[(pi, ci)] = d
                return t

            pq_t = load_pack(0, pq)
            pk_t = load_pack(1, pk)
            pv_t = load_pack(2, pv)

            # Q/K projections -> [feat, n] f32r tiles with bias folded in.
            qk_tiles = {}

            def qk_projection(t, f):
                pack_t, bcol = [(pq_t, 0), (pk_t, 4)][t]
                pt_out = projp.tile([128, N], F32R,
                                    tag=("qproj" if t == 0 else "kproj"))
                for nh in range(2):
                    ps = psump.tile([128, 512], F32, tag="proj")
                    for k in range(KT):
                        nc.tensor.matmul(
                            ps,
                            pack_t[:, k, 128 * f:128 * (f + 1)],
                            pack_t[:, k, FEAT + 512 * nh:FEAT + 512 * (nh + 1)],
                            start=(k == 0),
                            stop=(k == KT - 1),
                        )
                    nc.vector.tensor_scalar_add(
                        out=pt_out[:, 512 * nh:512 * (nh + 1)], in0=ps,
                        scalar1=bqk_t[:, bcol + f:bcol + f + 1],
                    )
                qk_tiles[(t, f)] = pt_out

            def v_projection(nts):
                for nt in nts:
                    vp = voutp.tile([128, FEAT], F32, tag="vproj")
                    ps = psump.tile([128, 512], F32, tag="proj")
                    for k in range(KT):
                        nc.tensor.matmul(
                            ps,
                            pv_t[:, k, FEAT + 128 * nt:FEAT + 128 * (nt + 1)],
                            pv_t[:, k, 0:FEAT],
                            start=(k == 0),
                            stop=(k == KT - 1),
                        )
                    nc.vector.scalar_tensor_tensor(
                        out=vp, in0=ps, scalar=1.0, in1=bv_t,
                        op0=mybir.AluOpType.mult, op1=mybir.AluOpType.add)
                    nc.sync.dma_start(
                        out=o[128 * nt:128 * (nt + 1), :], in_=vp)

            # Scores + softmax per head pair j (local heads 2j, 2j+1).
            # All Q projections run upfront (only need the first pack); the
            # K projection for pair j is issued right before pair j's scores
            # so softmax starts as soon as K f0 is ready instead of after
            # all of K.  16-matmul K blocks hide in the softmax backlog.
            inv_scale = 1.0 / SCALE
            for f in range(FEAT // 128):
                qk_projection(0, f)
            for f in range(FEAT // 128):
                qk_projection(1, f)
            for j in range(4):
                if j == 2:
                    v_projection(range(0, 4))
                elif j == 3:
                    v_projection(range(4, 8))
                qt = qk_tiles[(0, j)]
                kt_ = qk_tiles[(1, j)]
                for m in range(8):
                    at_t = attnp.tile([128, 2 * N], F32, tag="attn")
                    sm = sumsp.tile([128, 4], F32, tag="sums")
                    for hh in range(2):
                        lhsT = qt[64 * hh:64 * (hh + 1), 128 * m:128 * (m + 1)]
                        ps = psumsc.tile([128, N], F32, tag="sc")
                        for lh in range(2):
                            nc.tensor.matmul(
                                ps[:, 512 * lh:512 * (lh + 1)], lhsT,
                                kt_[64 * hh:64 * (hh + 1), 512 * lh:512 * (lh + 1)],
                                start=True, stop=True,
                            )
                        nc.scalar.activation(
                            out=at_t[:, N * hh:N * (hh + 1)],
                            in_=ps,
                            func=AF.Exp,
                            scale=inv_scale,
                            accum_out=sm[:, hh:hh + 1],
                        )
                    for hh in range(2):
                        nc.vector.reciprocal(sm[:, 2 + hh:3 + hh],
                                             sm[:, hh:hh + 1])
                        nc.vector.tensor_scalar_mul(
                            at_t[:, N * hh:N * (hh + 1)],
                            at_t[:, N * hh:N * (hh + 1)],
                            sm[:, 2 + hh:3 + hh])
                    out_ap = attn[2 * j:2 * j + 2,
                                  128 * m:128 * (m + 1), :].transpose([1, 0, 2])
                    nc.sync.dma_start(
                        out=out_ap,
                        in_=at_t[:].rearrange("p (h n) -> p h n", h=2))

    nc.compile()
    return nc


def make_in_maps(q, k, v, Wq, bq, Wk, bk, Wv, bv):
    q, k, v = (np.asarray(x, np.float32) for x in (q, k, v))
    Wq, Wk, Wv = (np.asarray(x, np.float32) for x in (Wq, Wk, Wv))
    bq, bk, bv = (np.asarray(x, np.float32) for x in (bq, bk, bv))

    xT = {}
    for b in range(B):
        xT[("q", b)] = np.ascontiguousarray(q[b].T)
        xT[("k", b)] = np.ascontiguousarray(k[b].T)
        xT[("v", b)] = np.ascontiguousarray(v[b].T)
    WT = {}
    for s in range(2):
        WT[("q", s)] = np.ascontiguousarray(Wq[FEAT * s:FEAT * (s + 1), :].T)
        WT[("k", s)] = np.ascontiguousarray(Wk[FEAT * s:FEAT * (s + 1), :].T)
        WT[("v", s)] = np.ascontiguousarray(Wv[FEAT * s:FEAT * (s + 1), :].T)

    in_maps = []
    for c in range(NCORES):
        b, s = divmod(c, 2)
        m = {}
        for t, name in (("q", "pq"), ("k", "pk"), ("v", "pv")):
            m[name] = np.concatenate([WT[(t, s)], xT[(t, b)]], axis=1)
        bqk_arr = np.empty((128, 8), np.float32)
        for f in range(4):
            bqk_arr[:, f] = bq[FEAT * s + 128 * f:FEAT * s + 128 * (f + 1)]
            bqk_arr[:, 4 + f] = bk[FEAT * s + 128 * f:FEAT * s + 128 * (f + 1)]
        m["bqk"] = bqk_arr
        m["bvr"] = bv[FEAT * s:FEAT * (s + 1)].copy()
        in_maps.append(m)
    return in_maps


def gather(results):
    out = np.empty((B, H, N, D), np.float32)
    attn = np.empty((B, H, N, N), np.float32)
    for c in range(NCORES):
        b, s = divmod(c, 2)
        attn[b, HPC * s:HPC * (s + 1)] = results[c]["attn"]
        out[b, HPC * s:HPC * (s + 1)] = (
            results[c]["o"].reshape(N, HPC, D).transpose(1, 0, 2))
    return out, attn


def get_nc():
    global _cached_nc
    if _cached_nc is None:
        _cached_nc = _build()
    return _cached_nc


def kernel(q, k, v, Wq, bq, Wk, bk, Wv, bv):
    nc = get_nc()
    in_maps = make_in_maps(q, k, v, Wq, bq, Wk, bk, Wv, bv)
    res = run_bass_kernel_spmd(nc, in_maps, list(range(NCORES)))
    return gather(res.results)


# revision 23
# speedup vs baseline: 1.1634x; 1.0254x over previous
"""Trainium2 Bass kernel for nn_Attention (B=4, N=1024, E=1024, H=16).

Computation (matching the reference):
    Q = q @ Wq.T + bq ; K = k @ Wk.T + bk ; V = v @ Wv.T + bv   (per batch)
    attn = softmax(Q_h @ K_h^T / sqrt(E))  per head h           -> output 2
    out  = V_h * rowsum(attn) = V_h (softmax rows sum to 1)     -> output 1

Sharding: 8 cores = 4 batches x 2 head-halves. Core c handles batch c//2 and
heads 8*(c%2) .. 8*(c%2)+8 (a 512-wide slice of the projection features).

Per core:
  - one packed [Wt | xT] tensor per projection, loaded with casting DMAs
    (fp32 -> fp32r: full PE-rate matmuls at ~12-13 bit mantissa) in 2-k-tile
    chunks, lane-chained q->k->v so the Q pack lands at full bandwidth first,
  - Q/K projections into [feat, n] f32r SBUF tiles (bias via DVE on PSUM
    eviction),
  - per head pair / 128-row tile: scores into a 2-bank PSUM tile, one
    exp(x/32) pass on the Scalar engine with fused row-sum, reciprocal +
    normalize on the Vector engine, one 1MB DMA out,
  - V projection split across the last two head pairs, bias via broadcast
    add, DMA out.
"""

import numpy as np

import concourse.bass as bass
import concourse.mybir as mybir
import concourse.tile as tile
from concourse import bacc
from concourse.bass_utils import run_bass_kernel_spmd
from concourse.tile_rust import add_dep_helper

B, N, E, H = 4, 1024, 1024, 16
D = E // H            # 64
NCORES = 8
FEAT = E // 2         # 512 projection features per core
HPC = H // 2          # 8 heads per core
KT = E // 128         # 8 contraction tiles
SCALE = float(E) ** 0.5

F32 = mybir.dt.float32
F32R = mybir.dt.float32r
AF = mybir.ActivationFunctionType

_cached_nc = None


def _build():
    nc = bacc.Bacc("TRN2", target_bir_lowering=False, debug=False)
    pq = nc.declare_dram_parameter("pq", [E, FEAT + N], F32R, isOutput=False)
    pk = nc.declare_dram_parameter("pk", [E, FEAT + N], F32R, isOutput=False)
    pv = nc.declare_dram_parameter("pv", [E, FEAT + N], F32R, isOutput=False)
    bqk = nc.declare_dram_parameter("bqk", [128, 8], F32, isOutput=False)
    bvr = nc.declare_dram_parameter("bvr", [FEAT], F32, isOutput=False)
    attn = nc.declare_dram_parameter("attn", [HPC, N, N], F32, isOutput=True)
    o = nc.declare_dram_parameter("o", [N, FEAT], F32, isOutput=True)

    with tile.TileContext(nc) as tc:
        with tc.tile_pool(name="packs", bufs=2) as packp, \
             tc.tile_pool(name="proj", bufs=8) as projp, \
             tc.tile_pool(name="vout", bufs=3) as voutp, \
             tc.tile_pool(name="attnp", bufs=4) as attnp, \
             tc.tile_pool(name="small", bufs=1) as smallp, \
             tc.tile_pool(name="sums", bufs=12) as sumsp, \
             tc.tile_pool(name="psumproj", bufs=2, space="PSUM") as psump, \
             tc.tile_pool(name="psumsc", bufs=3, space="PSUM") as psumsc:

            bqk_t = smallp.tile([128, 8], F32, tag="bqk")
            nc.sync.dma_start(out=bqk_t, in_=bqk[:, :])
            bv_t = smallp.tile([128, FEAT], F32, tag="bv")
            bvr_ap = bvr[:]
            bv_bcast = bass.AP(
                tensor=bvr_ap.tensor,
                offset=bvr_ap.offset,
                ap=[[0, 128]] + [list(x) for x in bvr_ap.ap],
            )
            nc.sync.dma_start(out=bv_t, in_=bv_bcast)

            # Chained casting pack loads, 2-k-tile chunks: chunks of one pack
            # stream concurrently; pack i's chunk c waits on pack i-1's
            # chunk c, so earlier packs get full aggregate bandwidth.
            CH = 2
            chunk_dmas = {}

            def load_pack(pi, param):
                t = packp.tile([128, KT, FEAT + N], F32R, tag="pack")
                src = param.rearrange("(k p) m -> p k m", p=128)
                for ci, c0 in enumerate(range(0, KT, CH)):
                    d = nc.sync.dma_start(
                        out=t[:, c0:c0 + CH, :], in_=src[:, c0:c0 + CH, :])
                    if pi > 0:
                        add_dep_helper(d.ins, chunk_dmas[(pi - 1, ci)].ins,
                                       reason="pack order q->k->v per lane")
                    chunk_dmas[(pi, ci)] = d
                return t

            pq_t = load_pack(0, pq)
            pk_t = load_pack(1, pk)
            pv_t = load_pack(2, pv)

            # Q/K projections -> [feat, n] f32r tiles with bias folded in.
            qk_tiles = {}

            def qk_projection(t, f):
                pack_t, bcol = [(pq_t, 0), (pk_t, 4)][t]
                pt_out = projp.tile([128, N], F32R,
                                    tag=("qproj" if t == 0 else "kproj"))
                for nh in range(2):
                    ps = psump.tile([128, 512], F32, tag="proj")
                    for k in range(KT):
                        nc.tensor.matmul(
                            ps,
                            pack_t[:, k, 128 * f:128 * (f + 1)],
                            pack_t[:, k, FEAT + 512 * nh:FEAT + 512 * (nh + 1)],
                            start=(k == 0),
                            stop=(k == KT - 1),
                        )
                    nc.vector.tensor_scalar_add(
                        out=pt_out[:, 512 * nh:512 * (nh + 1)], in0=ps,
                        scalar1=bqk_t[:, bcol + f:bcol + f + 1],
                    )
                qk_tiles[(t, f)] = pt_out

            def v_projection(nts):
                for nt in nts:
                    vp = voutp.tile([128, FEAT], F32, tag="vproj")
                    ps = psump.tile([128, 512], F32, tag="proj")
                    for k in range(KT):
                        nc.tensor.matmul(
                            ps,
                            pv_t[:, k, FEAT + 128 * nt:FEAT + 128 * (nt + 1)],
                            pv_t[:, k, 0:FEAT],
                            start=(k == 0),
                            stop=(k == KT - 1),
                        )
                    nc.vector.scalar_tensor_tensor(
                        out=vp, in0=ps, scalar=1.0, in1=bv_t,
                        op0=mybir.AluOpType.mult, op1=mybir.AluOpType.add)
                    nc.sync.dma_start(
                        out=o[128 * nt:128 * (nt + 1), :], in_=vp)

            # Scores + softmax per head pair j (local heads 2j, 2j+1).
            # All Q projections run upfront (only need the first pack); the
            # K projection for pair j is issued right before pair j's scores
            # so softmax starts as soon as K f0 is ready instead of after
            # all of K.  16-matmul K blocks hide in the softmax backlog.
            inv_scale = 1.0 / SCALE
            for f in range(FEAT // 128):
                qk_projection(0, f)
            for f in range(FEAT // 128):
                qk_projection(1, f)
            for j in range(4):
                if j == 2:
                    v_projection(range(0, 4))
                elif j == 3:
                    v_projection(range(4, 8))
                qt = qk_tiles[(0, j)]
                kt_ = qk_tiles[(1, j)]
                for m in range(8):
                    at_t = attnp.tile([128, 2 * N], F32, tag="attn")
                    sm = sumsp.tile([128, 4], F32, tag="sums")
                    for hh in range(2):
                        lhsT = qt[64 * hh:64 * (hh + 1), 128 * m:128 * (m + 1)]
                        ps = psumsc.tile([128, N], F32, tag="sc")
                        for lh in range(2):
                            nc.tensor.matmul(
                                ps[:, 512 * lh:512 * (lh + 1)], lhsT,
                                kt_[64 * hh:64 * (hh + 1), 512 * lh:512 * (lh + 1)],
                                start=True, stop=True,
                            )
                        nc.scalar.activation(
                            out=at_t[:, N * hh:N * (hh + 1)],
                            in_=ps,
                            func=AF.Exp,
                            scale=inv_scale,
                            accum_out=sm[:, hh:hh + 1],
                        )
                    for hh in range(2):
                        nc.vector.reciprocal(sm[:, 2 + hh:3 + hh],
                                             sm[:, hh:hh + 1])
                        nc.vector.tensor_scalar_mul(
                            at_t[:, N * hh:N * (hh + 1)],
                            at_t[:, N * hh:N * (hh + 1)],
                            sm[:, 2 + hh:3 + hh])
                    out_ap = attn[2 * j:2 * j + 2,
                                  128 * m:128 * (m + 1), :].transpose([1, 0, 2])
                    nc.sync.dma_start(
                        out=out_ap,
                        in_=at_t[:].rearrange("p (h n) -> p h n", h=2))

    nc.compile()
    return nc


def make_in_maps(q, k, v, Wq, bq, Wk, bk, Wv, bv):
    q, k, v = (np.asarray(x, np.float32) for x in (q, k, v))
    Wq, Wk, Wv = (np.asarray(x, np.float32) for x in (Wq, Wk, Wv))
    bq, bk, bv = (np.asarray(x, np.float32) for x in (bq, bk, bv))

    xT = {}
    for b in range(B):
        xT[("q", b)] = np.ascontiguousarray(q[b].T)
        xT[("k", b)] = np.ascontiguousarray(k[b].T)
        xT[("v", b)] = np.ascontiguousarray(v[b].T)
    WT = {}
    for s in range(2):
        WT[("q", s)] = np.ascontiguousarray(Wq[FEAT * s:FEAT * (s + 1), :].T)
        WT[("k", s)] = np.ascontiguousarray(Wk[FEAT * s:FEAT * (s + 1), :].T)
        WT[("v", s)] = np.ascontiguousarray(Wv[FEAT * s:FEAT * (s + 1), :].T)

    in_maps = []
    for c in range(NCORES):
        b, s = divmod(c, 2)
        m = {}
        for t, name in (("q", "pq"), ("k", "pk"), ("v", "pv")):
            m[name] = np.concatenate([WT[(t, s)], xT[(t, b)]], axis=1)
        bqk_arr = np.empty((128, 8), np.float32)
        for f in range(4):
            bqk_arr[:, f] = bq[FEAT * s + 128 * f:FEAT * s + 128 * (f + 1)]
            bqk_arr[:, 4 + f] = bk[FEAT * s + 128 * f:FEAT * s + 128 * (f + 1)]
        m["bqk"] = bqk_arr
        m["bvr"] = bv[FEAT * s:FEAT * (s + 1)].copy()
        in_maps.append(m)
    return in_maps


def gather(results):
    out = np.empty((B, H, N, D), np.float32)
    attn = np.empty((B, H, N, N), np.float32)
    for c in range(NCORES):
        b, s = divmod(c, 2)
        attn[b, HPC * s:HPC * (s + 1)] = results[c]["attn"]
        out[b, HPC * s:HPC * (s + 1)] = (
            results[c]["o"].reshape(N, HPC, D).transpose(1, 0, 2))
    return out, attn


def get_nc():
    global _cached_nc
    if _cached_nc is None:
        _cached_nc = _build()
    return _cached_nc


def kernel(q, k, v, Wq, bq, Wk, bk, Wv, bv):
    nc = get_nc()
    in_maps = make_in_maps(q, k, v, Wq, bq, Wk, bk, Wv, bv)
    res = run_bass_kernel_spmd(nc, in_maps, list(range(NCORES)))
    return gather(res.results)
